# revision 20
# baseline (speedup 1.0000x reference)
"""Trainium2 Bass kernel for nn_AttnAggregator (GAT-style neighbor aggregation).

Reference computation:
    ep = embed_table @ W_proj.T                      # [N, 64]
    neigh = ep[padded_neighs]                        # [B, L, 64]
    scores = leaky_relu(ep[dst]@a_dst) + leaky_relu(neigh@a_src)
    attn = masked_softmax(scores, axis=L)
    out = sum_l attn * neigh                         # [B, 64]

Key algebraic fact: the dst term is constant along the softmax axis L, so it
cancels in the softmax — the output does not depend on dst_idx / a_dst.

Strategy (8 NeuronCores, memory-bound target):
  Launch 1 (projection, table-row-sharded): each core projects N/8 = 25000
    table rows to 64 features and also precomputes es = exp(leaky_relu(ep@a_src))
    per row, emitting an extended row [64 feats | es] = 65 f32. Host
    concatenates the 8 slices into the full extended table (a pure unshard).
    The table is cast f32->bf16 during the SWDGE load DMA (full f32 HBM read;
    bf16 matmul runs the PE at 4x the fp32 rate), rows are ordered p-major
    within each 2048-row block (row = B0 + 16p + j via strided lhsT slices) so
    each partition writes one contiguous 4160B run per block, and the es
    epilogue is batched to 4 DVE/ACT instructions per block. ~99us.
    (CPB=32 measured slower: 117us, PSUM double-buffer pressure.)
  Launch 2 (attention, batch-sharded): each core handles B/8 = 6250 nodes.
    Masked neighbors have exactly zero softmax weight, so the host compacts
    each node's unmasked neighbors to the front and the kernel only gathers
    up to each 128-node tile's max unmasked count (~26 of 50 columns after
    sorting nodes by count). Each neighbor column is one indirect DMA (the
    TRN2 indirect DMA supports exactly one row index per partition: 128 rows
    of 260B per instruction); then on-chip: w = es*mask, den = sum_l w,
    num = sum_l w*feat, out = num/den. Nodes are globally sorted by count
    and striped across cores so all cores' per-tile maxima align: 1263
    gather instructions vs the 1245 edge-count floor. The launch is pinned
    to the SWDGE architectural floor: ~994ns Q7 descriptor generation +
    ~310ns NX dispatch/join per indirect DMA = ~1.46us x 1263 ~ 1.84ms;
    gather bytes (15% SDMA) and DVE compute (14%) hide entirely under it.
    Measured floor notes: dma_gather's ucode is ~9ns/idx (worse than
    indirect's ~1ns/idx + 994ns fixed), crashes above ~16K idxs, and
    multi-queue SWDGE does not parallelize descgen, so per-column indirect
    DMA is the best available gather primitive for this shape.
    Other dead ends measured this session: wide-output indirect DMA (K>=2
    rows per index) returns misaligned garbage on HW; bare back-to-back
    gathers with zero deps still run at ~1.46us/instr (dispatch floor);
    a [1,128] row-contiguous offset AP (to speed the Q7's cross-partition
    index reads) crashes the device; moving gathers into launch 1's shadow
    loses to node-aligned chunk padding; merging the launches loses the
    free host-side all-gather of ep between them.
    Measured: ~99us (launch 1) + ~1.85ms (launch 2), rel err ~2.9e-3
    (bf16 projection) vs the fp32 reference.
"""

import os
import sys

sys.path.insert(0, "/opt/trn_rl_repo")

import numpy as np

# ---- hardcoded problem shapes -------------------------------------------------
B, L, N = 50000, 50, 200000
D_IN, D_OUT = 256, 64
NCORES = 8
R = N // NCORES        # 25000 table rows per core (launch 1)
BN = B // NCORES       # 6250 batch nodes per core (launch 2)
EXT = D_OUT + 1        # 65: [64 projected feats | exp(leaky_relu(score))]
P = 128

_CACHE = {}
LAST_PERF = []         # filled when KERNEL_TRACE=1: list of BassKernelResults


def _build_proj():
    import concourse.bass as bass
    from concourse import bacc, mybir
    from concourse.tile import TileContext
    from contextlib import ExitStack

    F32 = mybir.dt.float32
    BF16 = mybir.dt.bfloat16
    nc = bacc.Bacc("TRN2", target_bir_lowering=False)
    tT = nc.dram_tensor("tT", [D_IN, R], F32, kind="ExternalInput")
    wT = nc.dram_tensor("wT", [D_IN, D_OUT], F32, kind="ExternalInput")
    a = nc.dram_tensor("a", [P, D_OUT], F32, kind="ExternalInput")
    ep = nc.dram_tensor("ep", [R, EXT], F32, kind="ExternalOutput")

    CPB = 16  # chunks (of 128 table rows) per PSUM block; 16*64 = 1024 f32 = 2 banks
    BLK = P * CPB

    with TileContext(nc) as tc, ExitStack() as ctx:
        singles = ctx.enter_context(tc.tile_pool(name="singles", bufs=1))
        tpool = ctx.enter_context(tc.tile_pool(name="tpool", bufs=6))
        stpool = ctx.enter_context(tc.tile_pool(name="stpool", bufs=3))
        bcpool = ctx.enter_context(tc.tile_pool(name="bcpool", bufs=3))
        espool = ctx.enter_context(tc.tile_pool(name="espool", bufs=3))
        spool = ctx.enter_context(tc.tile_pool(name="spool", bufs=4))
        psum = ctx.enter_context(tc.tile_pool(name="psum", bufs=2, space="PSUM"))

        w_ld = singles.tile([P, 2, D_OUT], F32)
        nc.sync.dma_start(out=w_ld[:], in_=wT.rearrange("(k p) n -> p k n", p=P))
        # stage weights through DVE (f32 -> bf16) so matmuls never wait on
        # the weight DMA; bf16 matmul runs the PE at 4x the fp32 rate.
        w_sb = singles.tile([P, 2, D_OUT], BF16)
        nc.vector.tensor_copy(out=w_sb[:], in_=w_ld[:])
        a_sb = singles.tile([P, 1, D_OUT], F32)
        nc.sync.dma_start(out=a_sb[:, 0, :], in_=a[:, :])
        tTr = tT.rearrange("(k p) r -> p k r", p=P)

        blocks = list(range(0, R, BLK))
        if R % BLK:
            blocks = blocks[-1:] + blocks[:-1]  # partial tail first
        for B0 in blocks:
            wcols = min(BLK, R - B0)
            nj = (wcols + P - 1) // P
            full = wcols == BLK
            # SWDGE cast-DMA: reads the full f32 table from HBM, lands bf16
            # in SBUF (the Pool engine is otherwise idle in this launch).
            tt = tpool.tile([P, 2, BLK], BF16)
            nc.gpsimd.dma_start(out=tt[:, :, :wcols], in_=tTr[:, :, B0 : B0 + wcols])
            # Full blocks order rows p-major (row = B0 + 8p + j, via strided
            # lhsT slices) so each partition's 8 output rows are one
            # contiguous 2080B HBM run: 128 big write descriptors per block
            # instead of 2048 small ones.
            ttv = tt.rearrange("p k (m j) -> p k m j", j=CPB)
            ps = psum.tile([P, CPB * D_OUT], F32, space="PSUM")
            for j in range(nj):
                if full:
                    lhs0 = ttv[:, 0, :, j]
                    lhs1 = ttv[:, 1, :, j]
                    cw = P
                else:
                    c0 = B0 + j * P
                    cw = min(P, R - c0)
                    lhs0 = tt[:, 0, j * P : j * P + cw]
                    lhs1 = tt[:, 1, j * P : j * P + cw]
                nc.tensor.matmul(
                    ps[:cw, j * D_OUT : (j + 1) * D_OUT],
                    lhs0,
                    w_sb[:, 0, :],
                    start=True,
                    stop=False,
                )
                nc.tensor.matmul(
                    ps[:cw, j * D_OUT : (j + 1) * D_OUT],
                    lhs1,
                    w_sb[:, 1, :],
                    start=False,
                    stop=True,
                )
            # assemble [feats | es] rows in one tile so the block writes back
            # as a single contiguous-per-partition DMA
            WH = bcpool.tile([P, CPB, EXT], F32)
            nc.vector.tensor_copy(
                out=WH[:, :nj, 0:D_OUT],
                in_=ps[:, 0 : nj * D_OUT].rearrange("p (j d) -> p j d", d=D_OUT),
            )
            # batched es epilogue: one mult, one reduce, one leaky, one exp
            scr = spool.tile([P, CPB, D_OUT], F32)
            nc.vector.tensor_tensor(
                out=scr[:, :nj, :],
                in0=WH[:, :nj, 0:D_OUT],
                in1=a_sb[:].to_broadcast([P, nj, D_OUT]),
                op=mybir.AluOpType.mult,
            )
            sc = spool.tile([P, CPB], F32)
            nc.vector.tensor_reduce(
                out=sc[:, :nj], in_=scr[:, :nj, :], axis=mybir.AxisListType.X,
                op=mybir.AluOpType.add,
            )
            ES = espool.tile([P, CPB], F32)
            nc.vector.scalar_tensor_tensor(
                out=ES[:, :nj],
                in0=sc[:, :nj],
                scalar=0.2,
                in1=sc[:, :nj],
                op0=mybir.AluOpType.mult,
                op1=mybir.AluOpType.max,
            )
            nc.scalar.activation(
                out=WH[:, 0:nj, D_OUT:EXT].rearrange("p j o -> p (j o)"),
                in_=ES[:, 0:nj],
                func=mybir.ActivationFunctionType.Exp,
            )
            if full:
                nc.sync.dma_start(
                    out=ep[B0 : B0 + BLK, :].rearrange("(p j) e -> p j e", p=P),
                    in_=WH[:, :, :],
                )
            else:
                for j in range(nj):
                    c0 = B0 + j * P
                    cw = min(P, R - c0)
                    nc.sync.dma_start(
                        out=ep[c0 : c0 + cw, :], in_=WH[:cw, j, :]
                    )
    return nc


def _build_attn(tile_counts=None):
    import concourse.bass as bass
    from concourse import bacc, mybir
    from concourse.tile import TileContext
    from contextlib import ExitStack

    F32 = mybir.dt.float32
    I32 = mybir.dt.int32
    if tile_counts is None:
        tile_counts = [L] * ((BN + P - 1) // P)
    nc = bacc.Bacc("TRN2", target_bir_lowering=False)
    ep = nc.dram_tensor("ep", [N, EXT], F32, kind="ExternalInput")
    idx = nc.dram_tensor("idx", [BN, L], I32, kind="ExternalInput")
    mkf = nc.dram_tensor("mkf", [BN, L], F32, kind="ExternalInput")
    out = nc.dram_tensor("out", [BN, D_OUT], F32, kind="ExternalOutput")

    with TileContext(nc) as tc, ExitStack() as ctx:
        ipool = ctx.enter_context(tc.tile_pool(name="ipool", bufs=8))
        gpool = ctx.enter_context(tc.tile_pool(name="gpool", bufs=6))
        wfpool = ctx.enter_context(tc.tile_pool(name="wfpool", bufs=4))
        spool = ctx.enter_context(tc.tile_pool(name="spool", bufs=6))
        opool = ctx.enter_context(tc.tile_pool(name="opool", bufs=4))

        for ti, t0 in enumerate(range(0, BN, P)):
            p = min(P, BN - t0)
            Lc = tile_counts[ti]
            it = ipool.tile([P, L], I32)
            nc.sync.dma_start(out=it[:p, :Lc], in_=idx[t0 : t0 + p, 0:Lc])
            mt = ipool.tile([P, L], F32)
            nc.sync.dma_start(out=mt[:p, :Lc], in_=mkf[t0 : t0 + p, 0:Lc])
            G = gpool.tile([P, L, EXT], F32)
            # HW indirect DMA supports exactly one index per partition, so
            # gather one neighbor column (128 rows) per instruction. Columns
            # beyond this tile's max unmasked-neighbor count are skipped
            # entirely (host compacts unmasked neighbors to the front).
            for l in range(Lc):
                nc.gpsimd.indirect_dma_start(
                    out=G[:p, l, :],
                    out_offset=None,
                    in_=ep[:, :],
                    in_offset=bass.IndirectOffsetOnAxis(ap=it[:p, l : l + 1], axis=0),
                )
            w = spool.tile([P, L], F32)
            den = spool.tile([P, 1], F32)
            nc.vector.tensor_tensor(
                out=w[:p, :Lc], in0=G[:p, :Lc, D_OUT], in1=mt[:p, :Lc],
                op=mybir.AluOpType.mult,
            )
            nc.vector.tensor_reduce(
                out=den[:p], in_=w[:p, :Lc], axis=mybir.AxisListType.X,
                op=mybir.AluOpType.add,
            )
            WF = wfpool.tile([P, L, D_OUT], F32)
            wb = w[:p, :Lc].to_broadcast([p, Lc, D_OUT])
            nc.vector.tensor_tensor(
                out=WF[:p, :Lc, :], in0=G[:p, :Lc, 0:D_OUT], in1=wb,
                op=mybir.AluOpType.mult,
            )
            num = spool.tile([P, D_OUT], F32)
            nc.vector.tensor_reduce(
                out=num[:p],
                in_=WF[:p, :Lc, :].rearrange("p l d -> p d l"),
                axis=mybir.AxisListType.X,
                op=mybir.AluOpType.add,
            )
            r = spool.tile([P, 1], F32)
            nc.vector.reciprocal(out=r[:p], in_=den[:p])
            ot = opool.tile([P, D_OUT], F32)
            rb = r[:p].to_broadcast([p, D_OUT])
            nc.vector.tensor_tensor(
                out=ot[:p], in0=num[:p], in1=rb, op=mybir.AluOpType.mult
            )
            nc.sync.dma_start(out=out[t0 : t0 + p, :], in_=ot[:p])
    return nc


def _get_nc(key, builder):
    if key not in _CACHE:
        nc = builder()
        nc.finalize()  # Bacc.finalize runs wait-splitting/legalization passes
        _CACHE[key] = nc
    return _CACHE[key]


def kernel(
    padded_neighs,
    mask,
    dst_idx,
    embed_table,
    W_proj,
    a_src,
    a_dst,
):
    from concourse.bass_utils import run_bass_kernel_spmd

    del dst_idx, a_dst  # constant along softmax axis -> cancels exactly

    trace = bool(int(os.environ.get("KERNEL_TRACE", "0")))
    LAST_PERF.clear()

    padded_neighs = np.asarray(padded_neighs, dtype=np.int32)
    mask = np.asarray(mask, dtype=bool)
    # Masked neighbors get exactly zero softmax weight (the reference masks
    # with -1e9 -> exp underflows to 0), so skipping them is exact. Compact
    # each node's unmasked neighbors to the front; the kernel then only
    # gathers up to each tile's max unmasked count.
    order = np.argsort(~mask, axis=1, kind="stable")
    padded_neighs = np.ascontiguousarray(np.take_along_axis(padded_neighs, order, axis=1))
    maskf = np.ascontiguousarray(
        np.take_along_axis(mask, order, axis=1).astype(np.float32)
    )
    # Sort ALL nodes by descending unmasked count and stripe them across
    # cores (global rank r -> core r % 8). Every core's sorted count
    # sequence is then nearly identical, so the per-tile-slot max over
    # cores adds ~nothing: sum(tile_counts) drops from ~1272 to ~1250
    # (floor = edges/128 ~ 1246). Pure row reordering: inputs permuted
    # here, outputs un-permuted below.
    counts = mask.sum(axis=1)  # [B]
    glob_order = np.argsort(-counts, kind="stable")
    core_nodes = [glob_order[c::NCORES] for c in range(NCORES)]  # each [BN], desc
    tile_counts = tuple(
        max(1, int(max(counts[core_nodes[c][t0]] for c in range(NCORES))))
        for t0 in range(0, BN, P)
    )
    tT = np.ascontiguousarray(np.asarray(embed_table, dtype=np.float32).T)
    wT = np.ascontiguousarray(np.asarray(W_proj, dtype=np.float32).T)
    a = np.ascontiguousarray(
        np.tile(np.asarray(a_src, dtype=np.float32)[None, :], (P, 1))
    )

    core_ids = list(range(NCORES))

    # ---- launch 1: projection (table rows sharded) ---------------------------
    nc1 = _get_nc("proj", _build_proj)
    in1 = [
        {
            "tT": np.ascontiguousarray(tT[:, c * R : (c + 1) * R]),
            "wT": wT,
            "a": a,
        }
        for c in core_ids
    ]
    res1 = run_bass_kernel_spmd(nc1, in1, core_ids=core_ids, trace=trace)
    ep = np.concatenate([r["ep"] for r in res1.results], axis=0)  # [N, EXT]

    # ---- launch 2: gather + attention (batch nodes sharded) ------------------
    nc2 = _get_nc(("attn", tile_counts), lambda: _build_attn(list(tile_counts)))
    in2 = [
        {
            "ep": ep,
            "idx": np.ascontiguousarray(padded_neighs[core_nodes[c]]),
            "mkf": np.ascontiguousarray(maskf[core_nodes[c]]),
        }
        for c in core_ids
    ]
    res2 = run_bass_kernel_spmd(nc2, in2, core_ids=core_ids, trace=trace)
    out = np.empty((B, D_OUT), dtype=np.float32)
    for c in core_ids:
        out[core_nodes[c]] = res2.results[c]["out"]

    if trace:
        LAST_PERF.extend([res1, res2])
    return np.ascontiguousarray(out, dtype=np.float32)

